# revision 20
# baseline (speedup 1.0000x reference)
"""Trainium2 Bass kernel for nn_Attention_23699629539900.

Data-parallel over batch: 8 cores, one batch element each, no collectives.

v2 design (p-state aware): the PE array only reaches its 2.4GHz p-state after
~3us of *continuous* execution; any stall drops it to 1.2/0.65GHz. The
baseline serialized dots->exp->AV per key-chunk, so attention ran de-clocked.
Here all 16 attention units (8 std heads + 8 ctx heads) form one stream, and
every other matmul in the model (qkv, v, both style vectorizers, mlp, w_out)
is emitted as *fill* between attention matmuls via a generator pump, so the
PE never idles while the scalar engine computes the softmax exps.

Engine split: ACT = exp stream only (+ tail Gelu/Identity; exp/identity share
one ACT table so no mid-stream table loads). DVE = evictions, lrelu, l2norms
(Newton rsqrt with the 0x5f3759df seed - no ACT Sqrt table switch),
softmax divides via reciprocal_approx_fast. GpSimd = cv-final evictions.

PSUM (8 banks): D dots tiles 2x[128,1024] (4), av accumulators 2x[128,512]
(2), fill GEMM tiles 2x[128,512] (2). E/v/cvh are bf16 (same PE rate,
half SBUF).
"""
import numpy as np
from contextlib import ExitStack

import concourse.bass as bass
import concourse.tile as tile
from concourse import bacc, mybir
from concourse.bass_utils import run_bass_kernel_spmd

F32 = mybir.dt.float32
F32R = mybir.dt.float32r
BF16 = mybir.dt.bfloat16
I32 = mybir.dt.int32
AF = mybir.ActivationFunctionType
ALU = mybir.AluOpType

B, N, D = 8, 1024, 512
H, DH = 8, 64
CH, CD = 6, 64
ID = H * DH
SCALE = DH ** -0.5
NT = N // 512          # query halves
KC = 8                 # key chunks of 128
MAGIC = 0x5F3759DF
# debug toggles for HW bisection
USE_NEWTON_RSQRT = [True]
USE_RECIP_FAST = [False]
USE_TTR = [False]
# column offsets in the packed "smallcols" input
COLS = {"nlb": 0, "bout": 4, "negsub": 8,
        "ckb0": 12, "ckb1": 16, "ckb2": 20, "cvb0": 24, "cvb1": 28,
        "magic": 32}
SCW = 36


def _declare_inputs(nc):
    t = {}
    def inp(name, shape, dt=F32R):
        t[name] = nc.dram_tensor(name, list(shape), dt, kind="ExternalInput").ap()
    inp("xT", (D, N))
    inp("ckT", (CH * CD, N))
    inp("cvT", (CH * CD, N))
    inp("wqkvT", (D, 3 * ID))
    inp("ckw0T", (CH * CD, ID)); inp("ckw1T", (ID, ID)); inp("ckw2T", (ID, ID))
    inp("cvw0T", (CH * CD, ID)); inp("cvw1T", (ID, ID)); inp("cvw2T", (ID, ID))
    inp("nlwT", (ID, ID)); inp("woutT", (D, ID))
    inp("rows", (1, 512))             # cvb2 row (free-direction bias)
    inp("smallcols", (128, SCW), F32)  # bias/negsub columns
    inp("ones_bf", (128, 512), BF16)
    inp("ones_row", (1, 128))  # denominator ones for v_av/cvh_av
    t["outT"] = nc.dram_tensor("outT", [D, N], F32, kind="ExternalOutput").ap()
    return t


def build_nc():
    _setup_act_tables()
    nc = bacc.Bacc("TRN2", target_bir_lowering=False, debug=False, num_devices=8)
    t = _declare_inputs(nc)

    with tile.TileContext(nc) as tc, ExitStack() as ctx:
        const = ctx.enter_context(tc.tile_pool(name="const", bufs=1))
        big = ctx.enter_context(tc.tile_pool(name="big", bufs=1))
        epool = ctx.enter_context(tc.tile_pool(name="ep", bufs=3))
        rpool = ctx.enter_context(tc.tile_pool(name="rp", bufs=3))
        psD = ctx.enter_context(tc.tile_pool(name="psD", bufs=2, space="PSUM"))
        psA = ctx.enter_context(tc.tile_pool(name="psA", bufs=2, space="PSUM"))
        psF = ctx.enter_context(tc.tile_pool(name="psF", bufs=2, space="PSUM"))

        # ---------------- constants ----------------
        smallcols = const.tile([128, SCW], F32)
        nc.sync.dma_start(smallcols[:], t["smallcols"][:])
        rows = const.tile([1, 512], F32R)
        nc.sync.dma_start(rows[:], t["rows"][:])
        ones1 = const.tile([1, 128], F32R)
        nc.sync.dma_start(ones1[:], t["ones_row"][:])
        # fp32 column whose bit pattern is the rsqrt seed constant 0x5F3759DF
        magic = smallcols[:, COLS["magic"]:COLS["magic"] + 4]
        s_ck = const.tile([128, 4], F32, name="s_ck")
        s_cv = const.tile([128, 4], F32, name="s_cv")
        s_mlp = const.tile([128, 4], F32, name="s_mlp")
        nr_t = const.tile([128, 4], F32, name="nr_t")
        nr_u = const.tile([128, 4], F32, name="nr_u")
        bcol = lambda nm: smallcols[:, COLS[nm]:COLS[nm] + 4]
        nlbc = bcol("nlb"); boutc = bcol("bout"); negsub = bcol("negsub")

        def rsqrt_inplace(s, k):
            # s[:, 0:k] := 1/sqrt(max(s, eps)); Newton w/ quake seed, DVE only
            sv = s[:, 0:k]
            nc.vector.tensor_scalar_max(sv, sv, 1e-24)
            if not USE_NEWTON_RSQRT[0]:
                nc.scalar.activation(sv, sv, AF.Sqrt, bias=0.0, scale=1.0)
                nc.vector.reciprocal(sv, sv)
                return
            tv = nr_t[:, 0:k]; uv = nr_u[:, 0:k]
            nc.vector.tensor_scalar(tv.bitcast(I32), sv.bitcast(I32), 1, None,
                                    op0=ALU.logical_shift_right)
            nc.vector.tensor_tensor(out=tv.bitcast(I32),
                                    in0=magic[:, 0:k].bitcast(I32),
                                    in1=tv.bitcast(I32), op=ALU.subtract)
            for _ in range(3):
                nc.vector.tensor_tensor(out=uv, in0=sv, in1=tv, op=ALU.mult)
                nc.vector.tensor_tensor(out=uv, in0=uv, in1=tv, op=ALU.mult)
                nc.vector.tensor_scalar(uv, uv, -0.5, 1.5,
                                        op0=ALU.mult, op1=ALU.add)
                nc.vector.tensor_tensor(out=tv, in0=tv, in1=uv, op=ALU.mult)
            nc.vector.tensor_copy(sv, tv)

        def sumsq(src_ap, accum_ap):
            if USE_TTR[0]:
                nc.vector.tensor_tensor_reduce(
                    out=sq_scr[:], in0=src_ap, in1=src_ap, scale=1.0,
                    scalar=0.0, op0=ALU.mult, op1=ALU.add, accum_out=accum_ap)
            else:
                nc.vector.tensor_tensor(out=sq_scr[:], in0=src_ap, in1=src_ap,
                                        op=ALU.mult)
                nc.vector.reduce_sum(accum_ap, sq_scr[:],
                                     axis=mybir.AxisListType.X)

        def recip(r_ap, lz_ap, src_ap):
            # 1/Z = exp(-ln Z) on ACT; ln+exp live in one ACT table so the
            # softmax exp stream is never interrupted by a table load
            nc.scalar.activation(lz_ap, src_ap, AF.Ln, bias=0.0, scale=1.0)
            nc.scalar.activation(r_ap, lz_ap, AF.Exp, bias=0.0, scale=-1.0)

        # ---------------- long-lived tiles (tag-chained) ----------------
        xT = big.tile([128, 4, N], F32R, tag="xT", name="xT")
        wqkv = big.tile([128, 4, 3 * ID], F32R, tag="wqkv", name="wqkv")
        q = big.tile([128, 4, N], F32R, tag="q", name="q")
        k = big.tile([128, 4, N], F32R, tag="k", name="k")
        v_av = big.tile([128, KC, H, 128], BF16, tag="vav", name="v_av")
        outT_std = big.tile([128, 4, N], F32R, tag="ostd", name="outT_std")
        ck_in = big.tile([128, 3, N], F32R, tag="cin", name="ck_in")
        ckw0 = big.tile([128, 3, ID], F32R, tag="w0", name="ckw0")
        ckw1 = big.tile([128, 4, ID], F32R, tag="w1", name="ckw1")
        ckw2 = big.tile([128, 4, ID], F32R, tag="w2", name="ckw2")
        y0 = big.tile([128, 4, N], F32R, tag="y0", name="y0")
        y1 = big.tile([128, 4, N], F32R, tag="y1", name="y1")
        sq_scr = big.tile([128, N], F32, tag="scr", name="sq_scr")

        # ---------------- input DMAs (arrival-ordered) ----------------
        xT_r = t["xT"].rearrange("(c p) n -> p c n", p=128)
        nc.gpsimd.dma_start(xT[:, :, 0:512], xT_r[:, :, 0:512])
        nc.gpsimd.dma_start(xT[:, :, 512:1024], xT_r[:, :, 512:1024])
        wq_r = t["wqkvT"].rearrange("(c p) f -> p c f", p=128)
        def wslice(m):
            s = slice(m * 128, (m + 1) * 128)
            nc.sync.dma_start(wqkv[:, :, s], wq_r[:, :, s])
        for m in (4, 0, 8, 9, 10, 11, 5, 1, 6, 2, 7, 3):
            wslice(m)
        # ones into v_av denominator halves
        ones_r = t["ones_bf"].rearrange("p (h d) -> p h d", h=H)
        for kc in range(KC):
            nc.sync.dma_start(v_av[:, kc, :, 64:128], ones_r[:])
        nc.sync.dma_start(ck_in[:], t["ckT"].rearrange("(c p) n -> p c n", p=128))
        nc.sync.dma_start(ckw0[:], t["ckw0T"].rearrange("(c p) f -> p c f", p=128))
        nc.sync.dma_start(ckw1[:], t["ckw1T"].rearrange("(c p) f -> p c f", p=128))
        nc.sync.dma_start(ckw2[:], t["ckw2T"].rearrange("(c p) f -> p c f", p=128))

        # ---------------- eviction helpers ----------------
        def lrelu_evict(eng, dst, ps):
            # dst = max(ps, 0.2*ps) in 2 DVE/gpsimd passes
            eng.tensor_scalar_mul(dst, ps, 0.2)
            d2 = dst.bitcast(F32) if dst.dtype == F32R else dst
            eng.tensor_tensor(out=dst, in0=ps, in1=d2, op=ALU.max)

        # ---------------- fill generators (yield once per PE matmul) ----
        def g_qkv(ms):
            for m in ms:
                dst = q if m < 4 else k
                for qt in range(NT):
                    s = slice(qt * 512, (qt + 1) * 512)
                    ps = psF.tile([128, 512], F32, tag="fill", name=f"qkv{m}_{qt}")
                    for kk in range(4):
                        nc.tensor.matmul(ps[:], wqkv[:, kk, m * 128:(m + 1) * 128],
                                         xT[:, kk, s], start=(kk == 0), stop=(kk == 3),
                                         skip_group_check=True)
                        yield
                    nc.vector.tensor_copy(dst[:, m % 4, s], ps[:])

        def g_v(ts):
            for t8 in ts:
                ps = psF.tile([128, 512], F32, tag="fill", name=f"v{t8}")
                for kk in range(4):
                    nc.tensor.matmul(ps[:], xT[:, kk, t8 * 128:(t8 + 1) * 128],
                                     wqkv[:, kk, 2 * ID:3 * ID],
                                     start=(kk == 0), stop=(kk == 3),
                                     skip_group_check=True)
                    yield
                nc.vector.tensor_copy(v_av[:, t8, :, 0:64],
                                      ps[:].rearrange("p (h d) -> p h d", h=H))

        def norm_fold(cin, s_tile, w0):
            # per-channel 1/||.|| over tokens, folded into w0 columns (DVE)
            for c in range(3):
                sumsq(cin[:, c, :].bitcast(F32), s_tile[:, c:c + 1])
            rsqrt_inplace(s_tile, 3)
            for c in range(3):
                nc.vector.tensor_scalar_mul(w0[:, c, :], w0[:, c, :].bitcast(F32),
                                            s_tile[:, c:c + 1])

        def g_linear(w, nkk, rhs_get, out_write, tagn):
            for m in range(4):
                for qt in range(NT):
                    ps = psF.tile([128, 512], F32, tag="fill", name=f"{tagn}{m}_{qt}")
                    for kk in range(nkk):
                        nc.tensor.matmul(ps[:], w[:, kk, m * 128:(m + 1) * 128],
                                         rhs_get(kk, qt), start=(kk == 0),
                                         stop=(kk == nkk - 1), skip_group_check=True)
                        yield
                    out_write(m, qt, ps)

        lt = {}  # late tiles: allocated only once their tag predecessor's
                 # readers have all been EMITTED (tag-chain requirement)

        def g_ck_L0():
            norm_fold(ck_in, s_ck, ckw0)
            yield from g_linear(
                ckw0, 3, lambda kk, qt: ck_in[:, kk, qt * 512:(qt + 1) * 512],
                lambda m, qt, ps: lrelu_evict(nc.vector,
                                              y0[:, m, qt * 512:(qt + 1) * 512], ps[:]),
                "ckL0")
            # ck_in + ckw0 dead; cv twins chain onto them
            lt["cv_in"] = big.tile([128, 3, N], F32R, tag="cin", name="cv_in")
            nc.sync.dma_start(lt["cv_in"][:],
                              t["cvT"].rearrange("(c p) n -> p c n", p=128))
            lt["cvw0"] = big.tile([128, 3, ID], F32R, tag="w0", name="cvw0")
            nc.sync.dma_start(lt["cvw0"][:],
                              t["cvw0T"].rearrange("(c p) f -> p c f", p=128))

        def g_ck_L1():
            yield from g_linear(
                ckw1, 4, lambda kk, qt: y0[:, kk, qt * 512:(qt + 1) * 512],
                lambda m, qt, ps: lrelu_evict(nc.vector,
                                              y1[:, m, qt * 512:(qt + 1) * 512], ps[:]),
                "ckL1")
            lt["cvw1"] = big.tile([128, 4, ID], F32R, tag="w1", name="cvw1")
            nc.sync.dma_start(lt["cvw1"][:],
                              t["cvw1T"].rearrange("(c p) f -> p c f", p=128))

        def g_ck_L2():
            lt["ckh"] = big.tile([128, 4, N], F32R, tag="wqkv", name="ckh")
            yield from g_linear(
                ckw2, 4, lambda kk, qt: y1[:, kk, qt * 512:(qt + 1) * 512],
                lambda m, qt, ps: lrelu_evict(nc.vector,
                                              lt["ckh"][:, m, qt * 512:(qt + 1) * 512],
                                              ps[:]),
                "ckL2")
            lt["cvw2"] = big.tile([128, 4, ID], F32R, tag="w2", name="cvw2")
            nc.sync.dma_start(lt["cvw2"][:],
                              t["cvw2T"].rearrange("(c p) f -> p c f", p=128))

        def g_cv_L0():
            norm_fold(lt["cv_in"], s_cv, lt["cvw0"])
            yield from g_linear(
                lt["cvw0"], 3,
                lambda kk, qt: lt["cv_in"][:, kk, qt * 512:(qt + 1) * 512],
                lambda m, qt, ps: lrelu_evict(nc.vector,
                                              y0[:, m, qt * 512:(qt + 1) * 512], ps[:]),
                "cvL0")

        def g_cv_L1():
            yield from g_linear(
                lt["cvw1"], 4, lambda kk, qt: y0[:, kk, qt * 512:(qt + 1) * 512],
                lambda m, qt, ps: lrelu_evict(nc.vector,
                                              y1[:, m, qt * 512:(qt + 1) * 512], ps[:]),
                "cvL1")

        def g_cv_fin():
            # token-major final layer; bias added via K=1 ones-row matmul
            for kc in range(KC):
                ps = psF.tile([128, 512], F32, tag="fill", name=f"cvf{kc}")
                for kk in range(4):
                    nc.tensor.matmul(ps[:], y1[:, kk, kc * 128:(kc + 1) * 128],
                                     lt["cvw2"][:, kk, :], start=(kk == 0), stop=False,
                                     skip_group_check=True)
                    yield
                nc.tensor.matmul(ps[:], ones1[0:1, :], rows[0:1, :],
                                 start=False, stop=True, skip_group_check=True)
                yield
                lrelu_evict(nc.vector, cvh_av[:, kc, :, 0:64],
                            ps[:].rearrange("p (h d) -> p h d", h=H))

        # ---------------- fill pump ----------------
        fills = []
        def pump(n):
            while n > 0 and fills:
                try:
                    next(fills[0])
                    n -= 1
                except StopIteration:
                    fills.pop(0)

        # ---------------- attention unit ----------------
        def attn_unit(tag, h, lhsT_get, vav, out_write):
            hh, c = h % 2, h // 2
            avs = [psA.tile([128, 512], F32, tag="av", name=f"{tag}{h}av{qt}")
                   for qt in range(NT)]
            Es = []
            for kc in range(KC):
                Dt = psD.tile([128, 1024], F32, tag="D", name=f"{tag}{h}D{kc}")
                for qt in range(NT):
                    s = slice(qt * 512, (qt + 1) * 512)
                    nc.tensor.matmul(Dt[:, s], lhsT_get(h, kc),
                                     q[hh * 64:hh * 64 + 64, c, s],
                                     start=True, stop=True, skip_group_check=True)
                E = epool.tile([128, 1024], BF16, tag="E", name=f"{tag}{h}E{kc}")
                nc.scalar.activation(E[:], Dt[:], AF.Exp, bias=0.0, scale=SCALE)
                Es.append(E)
                pump(2)
                if kc > 0:
                    _av(avs, vav, h, kc - 1, Es[kc - 1], tag)
                pump(2)
            _av(avs, vav, h, KC - 1, Es[KC - 1], tag)
            for qt in range(NT):
                s = slice(qt * 512, (qt + 1) * 512)
                lz = rpool.tile([64, 512], F32, tag="r", name=f"{tag}{h}lz{qt}")
                r = rpool.tile([64, 512], F32, tag="r", name=f"{tag}{h}r{qt}")
                recip(r[:], lz[:], avs[qt][64:128, :])
                out_write(h, qt, s, avs[qt], r)

        def _av(avs, vav, h, kc, E, tag):
            for qt in range(NT):
                s = slice(qt * 512, (qt + 1) * 512)
                nc.tensor.matmul(avs[qt][:], vav[:, kc, h, :], E[:, s],
                                 start=(kc == 0), stop=(kc == KC - 1),
                                 skip_group_check=True)

        def std_write(h, qt, s, av, r):
            hh, c = h % 2, h // 2
            nc.vector.tensor_tensor(out=outT_std[hh * 64:hh * 64 + 64, c, s],
                                    in0=av[0:64, 0:512], in1=r[:],
                                    op=ALU.mult)

        def ctx_write(h, qt, s, av, r):
            hh, c = h % 2, h // 2
            octx = lt["outT_ctx"]
            nc.vector.tensor_tensor(out=octx[hh * 64:hh * 64 + 64, c, s],
                                    in0=av[0:64, 0:512], in1=r[:], op=ALU.mult)
            if hh == 1 and qt == NT - 1:
                # chunk c of outT_ctx complete -> mlp-norm sumsq
                sumsq(octx[:, c, :].bitcast(F32), s_mlp[:, c:c + 1])

        # ---------------- lead-in ----------------
        for _ in g_qkv([4, 0]):
            pass
        for _ in g_v([0, 1]):
            pass

        # cvh_av has a fresh tag; safe to allocate early (ones DMA overlaps)
        cvh_av = big.tile([128, KC, H, 128], BF16, tag="vav2", name="cvh_av")
        for kc in range(KC):
            nc.sync.dma_start(cvh_av[:, kc, :, 64:128], ones_r[:])

        fills.extend([g_v([2, 3, 4, 5, 6, 7]), g_qkv([5, 1]), g_ck_L0(),
                      g_qkv([6, 2]), g_ck_L1(), g_qkv([7, 3]), g_ck_L2(),
                      g_cv_L0(), g_cv_L1(), g_cv_fin()])

        # ---------------- attention units ----------------
        kT = lambda h, kc: k[(h % 2) * 64:(h % 2) * 64 + 64, h // 2,
                             kc * 128:(kc + 1) * 128]
        ckhT = lambda h, kc: lt["ckh"][(h % 2) * 64:(h % 2) * 64 + 64, h // 2,
                                       kc * 128:(kc + 1) * 128]
        for h in range(H):
            attn_unit("s", h, kT, v_av, std_write)

        # std emission done: k/v_av/cv_in/y-readers all emitted -> chain tiles
        lt["outT_ctx"] = big.tile([128, 4, N], F32R, tag="k", name="outT_ctx")
        nlw = big.tile([128, 4, ID], F32R, tag="vav", name="nlw")
        nc.sync.dma_start(nlw[:], t["nlwT"].rearrange("(c p) f -> p c f", p=128))
        wout = big.tile([128, 4, ID], F32R, tag="cin", name="wout")
        nc.sync.dma_start(wout[:], t["woutT"].rearrange("(c p) f -> p c f", p=128))

        for h in range(H):
            attn_unit("c", h, ckhT, cvh_av, ctx_write)
        pump(10 ** 9)  # drain any leftover fill

        # ---------------- tail: mlp, combine, w_out ----------------
        mlpT = big.tile([128, 4, N], F32R, tag="y0", name="mlpT")
        osb = big.tile([128, 4, N], F32, tag="xT", name="osb")
        rsqrt_inplace(s_mlp, 4)
        for c in range(4):
            nc.vector.tensor_scalar_mul(nlw[:, c, :], nlw[:, c, :].bitcast(F32),
                                        s_mlp[:, c:c + 1])
        for m in range(4):
            for qt in range(NT):
                s = slice(qt * 512, (qt + 1) * 512)
                ps = psF.tile([128, 512], F32, tag="fill", name=f"mlp{m}_{qt}")
                for kk in range(4):
                    nc.tensor.matmul(ps[:], nlw[:, kk, m * 128:(m + 1) * 128],
                                     lt["outT_ctx"][:, kk, s], start=(kk == 0),
                                     stop=(kk == 3), skip_group_check=True)
                nc.scalar.activation(mlpT[:, m, s], ps[:], AF.Gelu,
                                     bias=nlbc[:, m:m + 1], scale=1.0)
                # combine: mlpT = outT_std - sub_ratio*mlpT  (negsub = -sub)
                nc.vector.scalar_tensor_tensor(
                    out=mlpT[:, m, s], in0=mlpT[:, m, s].bitcast(F32),
                    scalar=negsub[:, m:m + 1],
                    in1=outT_std[:, m, s].bitcast(F32),
                    op0=ALU.mult, op1=ALU.add)
        outT_r = t["outT"].rearrange("(c p) n -> p c n", p=128)
        for m in range(4):
            for qt in range(NT):
                s = slice(qt * 512, (qt + 1) * 512)
                ps = psF.tile([128, 512], F32, tag="fill", name=f"wo{m}_{qt}")
                for kk in range(4):
                    nc.tensor.matmul(ps[:], wout[:, kk, m * 128:(m + 1) * 128],
                                     mlpT[:, kk, s], start=(kk == 0),
                                     stop=(kk == 3), skip_group_check=True)
                nc.vector.tensor_scalar_add(osb[:, m, s], ps[:],
                                            boutc[:, m:m + 1])
                nc.sync.dma_start(outT_r[:, m, s], osb[:, m, s])

    nc.compile()
    return nc


def make_in_maps(x, ck, cv, w_qkv, w_out, b_out,
                 ckw0, ckb0, ckw1, ckb1, ckw2, ckb2,
                 cvw0, cvb0, cvw1, cvb1, cvw2, cvb2,
                 nl_w, nl_b, sub_ratio):
    import ml_dtypes
    f32 = lambda a: np.ascontiguousarray(a, dtype=np.float32)
    rows = np.asarray(cvb2, np.float32).reshape(1, 512).copy()
    smallcols = np.zeros((128, SCW), np.float32)
    smallcols[:, 32:36] = np.uint32(0x5F3759DF).view(np.float32)
    for nm, arr in (("nlb", nl_b), ("bout", b_out),
                    ("ckb0", ckb0), ("ckb1", ckb1), ("ckb2", ckb2),
                    ("cvb0", cvb0), ("cvb1", cvb1)):
        smallcols[:, COLS[nm]:COLS[nm] + 4] = np.asarray(arr, np.float32).reshape(4, 128).T
    smallcols[:, 8:12] = -np.asarray(sub_ratio, np.float32).reshape(4, 128).T
    shared = {
        "wqkvT": f32(w_qkv.T),
        "ckw0T": f32(ckw0.T), "ckw1T": f32(ckw1.T), "ckw2T": f32(ckw2.T),
        "cvw0T": f32(cvw0.T), "cvw1T": f32(cvw1.T), "cvw2T": f32(cvw2.T),
        "nlwT": f32(nl_w.T), "woutT": f32(w_out.T),
        "rows": rows, "smallcols": smallcols,
        "ones_bf": np.ones((128, 512), ml_dtypes.bfloat16),
        "ones_row": np.ones((1, 128), np.float32),
    }
    in_maps = []
    for b in range(B):
        m = dict(shared)
        m["xT"] = f32(x[b].T)
        m["ckT"] = f32(ck[b].transpose(0, 2, 1).reshape(CH * CD, N))
        m["cvT"] = f32(cv[b].transpose(0, 2, 1).reshape(CH * CD, N))
        in_maps.append(m)
    return in_maps


def _setup_act_tables():
    """Reorder act_info.json so the table holding BOTH exp and ln comes
    first: the table-load pass prefers the first matching table, so the
    softmax exp stream and the ln/exp reciprocal share one table and no
    mid-stream ACT_TABLE_LOADs are emitted. Only the entry ORDER changes;
    walrus reads the same reordered file via BASS_ACT_ROOT_JSON_PATH, so
    ids stay consistent."""
    import json as _json, os as _os, tempfile as _tempfile
    if _os.environ.get("BASS_ACT_ROOT_JSON_PATH"):
        return
    from neuronxcc.driver.Job import Job as _Job
    from neuronxcc.driver.jobs.support.FindActInfo import (
        findActInfoFile as _find)
    srcp = _find(_Job.getPackageDir(), "gen3")
    info = _json.load(open(srcp))
    sets = info["act_func_sets"]
    idx = next(i for i, e in enumerate(sets)
               if "exp" in e["act"] and "ln" in e["act"])
    sets.insert(0, sets.pop(idx))
    # table entries reference sibling .bin files relative to act_info.json;
    # symlink the whole pwp dir next to the reordered json
    tdir = _tempfile.mkdtemp(prefix="actinfo_")
    srcdir = _os.path.dirname(srcp)
    for fn in _os.listdir(srcdir):
        if fn != "act_info.json":
            _os.symlink(_os.path.join(srcdir, fn), _os.path.join(tdir, fn))
    path = _os.path.join(tdir, "act_info.json")
    _json.dump(info, open(path, "w"))
    _os.environ["BASS_ACT_ROOT_JSON_PATH"] = path
    import concourse.bacc as _bacc_mod
    tabs = {e["name"]: {mybir.ActivationFunctionType.from_pwp(v)
                        for v in e["act"].keys()} for e in sets}
    _bacc_mod.get_activation_tables = lambda arch: tabs


_NC_CACHE = {}


def get_nc():
    if "nc" not in _NC_CACHE:
        _NC_CACHE["nc"] = build_nc()
    return _NC_CACHE["nc"]


def kernel(**inputs):
    inputs = {k: np.asarray(v) for k, v in inputs.items()}
    nc = get_nc()
    in_maps = make_in_maps(**inputs)
    res = run_bass_kernel_spmd(nc, in_maps, list(range(B)))
    out = np.empty((B, N, D), np.float32)
    for b in range(B):
        out[b] = res.results[b]["outT"].T
    return out
